# revision 2
# baseline (speedup 1.0000x reference)
"""Trainium2 Bass kernel for DigitConvolutionalModel (conv3x3 -> FC512 -> FC10).

Math: the 3x3 valid conv is linear, so  y_flat = x @ C  with C [784, 676]
holding conv_w values in a banded structure.  Then
    logits = relu(x @ (C @ W1) + b1) @ W2 + b2
The fold W1_eff = C @ W1 is computed on device (banded matmul over only
the nonzero blocks), then the big [2048, 784] @ [784, 512] matmul per
core, relu, and the [*, 512] @ [512, 10] head.  Data-parallel across 8
cores on the batch dim.

v2 layout: weights split across all three DMA rings (sync/scalar/gpsimd)
so the fold starts ~2us earlier; x rides the same rings AFTER the weights
(same-ring ordering replaces the old dummy-gating); per-superblock output
DMAs pull the final DMA completion (which gates the fixed teardown) a few
us earlier; the last superblock's FC10 is split in half-N pieces
interleaved right after the L1 stream so the closing dependency chain is
short.
"""

import numpy as np
import ml_dtypes

B = 16384
IMG = 28
K = 3
OUT = IMG - K + 1  # 26
M26 = OUT * OUT  # 676
Q = IMG * IMG  # 784
HID = 512
NCLS = 10

NCORES = 8
BL = B // NCORES  # 2048 rows per core
QT = 112  # q-tile height (partition dim), 7 tiles
NQT = Q // QT  # 7
SB = 512  # batch superblock (matmul N)
NSB = BL // SB  # 4
NHT = HID // 128  # 4
NMC = (M26 + 127) // 128  # 6 m-chunks
NWARM = 10  # dummy matmuls riding out the PE HAM ramp + weight DMA

TRACE = False  # set by test harness to capture an NTFF profile
_CACHE = {}

_BF16 = ml_dtypes.bfloat16


def _band_blocks():
    """Static nonzero block pattern of C^T [676, 784] against (mc, qt) tiling."""
    Cs = np.zeros((Q, M26), dtype=bool)
    ii, jj = np.meshgrid(np.arange(OUT), np.arange(OUT), indexing="ij")
    m = (OUT * ii + jj).ravel()
    for di in range(K):
        for dj in range(K):
            q = ((ii + di) * IMG + (jj + dj)).ravel()
            Cs[q, m] = True
    CT = Cs.T  # [676, 784]
    blocks = []
    for t in range(NQT):
        mcs = []
        for mc in range(NMC):
            rows = min(128, M26 - 128 * mc)
            if CT[128 * mc : 128 * mc + rows, QT * t : QT * (t + 1)].any():
                mcs.append(mc)
        blocks.append(mcs)
    return blocks


_BLOCKS = _band_blocks()
_PAIRS = [(t, mc) for t in range(NQT) for mc in _BLOCKS[t]]
NP_ = len(_PAIRS)  # 14

# ---- 3-ring split of the weight DMAs, in fold-consumption order ----
# ring A (sync): cmb pairs 0-4 + W1 chunks mc 0,1   -> covers fold t0..t2
# ring B (scalar): cmb pairs 5-9 + W1 chunks mc 2,3 -> fold t3,t4
# ring C (gpsimd, slower SWDGE): cmb pairs 10-13 + mc 4,5 -> fold t4..t6
_CM_SPLIT = [(0, 5), (5, 10), (10, NP_)]  # pair index ranges per ring
_W1_SPLIT = [(0, 2), (2, 4), (4, 6)]  # mc ranges per ring


def _build():
    import concourse.bacc as bacc
    import concourse.mybir as mybir
    import concourse.tile as tile

    f32 = mybir.dt.float32
    bf16 = mybir.dt.bfloat16
    AF = mybir.ActivationFunctionType

    nc = bacc.Bacc("TRN2", target_bir_lowering=False, debug=False)

    xt_d = nc.dram_tensor("xt", [Q, BL], bf16, kind="ExternalInput")
    cm_d = [
        nc.dram_tensor(f"cm{r}", [128, (hi - lo) * QT], bf16, kind="ExternalInput")
        for r, (lo, hi) in enumerate(_CM_SPLIT)
    ]
    w1_d = [
        nc.dram_tensor(f"w1{r}", [128, (hi - lo) * HID], bf16, kind="ExternalInput")
        for r, (lo, hi) in enumerate(_W1_SPLIT)
    ]
    b1_d = nc.dram_tensor("b1l", [128, NHT], f32, kind="ExternalInput")
    w2_d = nc.dram_tensor("w2l", [128, NHT * NCLS], bf16, kind="ExternalInput")
    b2_d = nc.dram_tensor("b2l", [NCLS, 1], f32, kind="ExternalInput")
    out_d = nc.dram_tensor("out", [NCLS, BL], f32, kind="ExternalOutput")

    # pair index -> (ring, slot within ring's cmb piece); mc -> (ring, slot)
    pair_loc = {}
    for r, (lo, hi) in enumerate(_CM_SPLIT):
        for p in range(lo, hi):
            pair_loc[p] = (r, p - lo)
    mc_loc = {}
    for r, (lo, hi) in enumerate(_W1_SPLIT):
        for mc in range(lo, hi):
            mc_loc[mc] = (r, mc - lo)

    with tile.TileContext(nc) as tc:
        with (
            tc.tile_pool(name="weights", bufs=1) as wp,
            tc.tile_pool(name="xin", bufs=1) as xp,
            tc.tile_pool(name="hid", bufs=1) as hp,
            tc.tile_pool(name="lgts", bufs=1) as lp,
            tc.tile_pool(name="psF", bufs=2, space="PSUM") as psF,
            tc.tile_pool(name="ps1", bufs=1, space="PSUM") as ps1p,
            tc.tile_pool(name="ps2", bufs=1, space="PSUM") as ps2p,
        ):
            rings = [nc.sync, nc.scalar, nc.gpsimd]

            # ---- PE warmup: dependency-light matmuls on scratch data ----
            # Issue as soon as the tiny memset lands; they ride out the PE
            # power ramp (half-rate for the first ~5us of activity) and the
            # weight-DMA prologue.  Results never read.
            scratch = wp.tile([128, HID], bf16, tag="scratch")
            nc.vector.memset(scratch[:], 0.0)
            warm = psF.tile([128, HID], f32, tag="ps")
            for i in range(NWARM):
                nc.tensor.matmul(
                    warm[:],
                    lhsT=scratch[:, :128],
                    rhs=scratch[:],
                    start=True,
                    stop=True,
                )

            # ---- weight DMAs: one cmb piece + one W1 piece per ring, in
            # fold-consumption order.  x transfers are issued on the same
            # rings AFTER the weights, so ring order (not dummy gating)
            # keeps the weight transfers at full aggregate bandwidth.
            cmb = []
            for r, (lo, hi) in enumerate(_CM_SPLIT):
                t_ = wp.tile([128, (hi - lo) * QT], bf16, tag=f"cmb{r}")
                rings[r].dma_start(out=t_[:], in_=cm_d[r][:, :])
                cmb.append(t_)
            w1p = []
            for r, (lo, hi) in enumerate(_W1_SPLIT):
                t_ = wp.tile([128, (hi - lo) * HID], bf16, tag=f"w1p{r}")
                rings[r].dma_start(out=t_[:], in_=w1_d[r][:, :])
                w1p.append(t_)
            b1 = wp.tile([128, NHT], f32, tag="b1")
            nc.gpsimd.dma_start(out=b1[:], in_=b1_d[:, :])
            w2 = wp.tile([128, NHT * NCLS], bf16, tag="w2")
            nc.gpsimd.dma_start(out=w2[:], in_=w2_d[:, :])
            b2 = wp.tile([NCLS, 1], f32, tag="b2")
            nc.gpsimd.dma_start(out=b2[:], in_=b2_d[:, :])

            # ---- x DMAs, consumption order, round-robin across rings.
            # s=0,1 as [112, 512] tiles; s=2,3 as [112, 1024] pair tiles.
            xsm, xw = {}, []
            ring_i = 0
            for s in range(2):
                for t in range(NQT):
                    xx = xp.tile([QT, SB], bf16, tag=f"x{s}_{t}")
                    rings[ring_i % 3].dma_start(
                        out=xx[:],
                        in_=xt_d[QT * t : QT * (t + 1), SB * s : SB * (s + 1)],
                    )
                    ring_i += 1
                    xsm[(s, t)] = xx
            for t in range(NQT):
                xx = xp.tile([QT, 2 * SB], bf16, tag=f"xw_{t}")
                rings[ring_i % 3].dma_start(
                    out=xx[:], in_=xt_d[QT * t : QT * (t + 1), 2 * SB : BL]
                )
                ring_i += 1
                xw.append(xx)

            def xslice(s, t):
                if s < 2:
                    return xsm[(s, t)][:]
                return xw[t][:, SB * (s - 2) : SB * (s - 1)]

            # ---- fold: W1_eff[q, h] = sum_m C^T[m, q] * W1[m, h] ----
            pair_idx = {pair: i for i, pair in enumerate(_PAIRS)}
            w1eff = []

            FOLD_SLOT = ["ps", "ps", "ps1_0", "ps1_1", "ps1_2", "ps1_3", "ps"]

            def fold_block(t):
                slot = FOLD_SLOT[t]
                pool = psF if slot == "ps" else ps1p
                ps = pool.tile([QT, HID], f32, tag=slot, name=f"foldps_{t}")
                mcs = _BLOCKS[t]
                for j, mc in enumerate(mcs):
                    rows = min(128, M26 - 128 * mc)
                    p = pair_idx[(t, mc)]
                    pr, pslot = pair_loc[p]
                    wr, wslot = mc_loc[mc]
                    nc.tensor.matmul(
                        ps[:],
                        lhsT=cmb[pr][:rows, QT * pslot : QT * (pslot + 1)],
                        rhs=w1p[wr][:rows, HID * wslot : HID * (wslot + 1)],
                        start=(j == 0),
                        stop=(j == len(mcs) - 1),
                    )
                we = wp.tile([QT, HID], bf16, tag=f"we{t}", name=f"we{t}")
                half = HID // 2
                nc.vector.tensor_copy(we[:, :half], ps[:, :half])
                nc.scalar.activation(we[:, half:], ps[:, half:], AF.Copy)
                w1eff.append(we)

            # ---- L1 + L2.  One uninterrupted PE stream; relus fire on
            # scalar/vector while the PE rolls on; each superblock's L2 runs
            # after the NEXT superblock's L1 so no relu gates the PE FIFO.
            # The logits leave per-superblock so the final DMA (which gates
            # the fixed teardown) completes as early as possible.
            hs_all = {}
            lg = lp.tile([NCLS, BL], f32, tag="lg")

            def l1_block(s):
                ps1s = [
                    ps1p.tile([128, SB], f32, tag=f"ps1_{ht}", name=f"ps1_{ht}")
                    for ht in range(NHT)
                ]
                if s == 0:
                    # t-outer: consume each x tile in DMA-arrival order
                    for t in range(NQT):
                        for ht in range(NHT):
                            nc.tensor.matmul(
                                ps1s[ht][:],
                                lhsT=w1eff[t][:, 128 * ht : 128 * (ht + 1)],
                                rhs=xslice(s, t),
                                start=(t == 0),
                                stop=(t == NQT - 1),
                            )
                for ht in range(NHT):
                    if s != 0:
                        for t in range(NQT):
                            nc.tensor.matmul(
                                ps1s[ht][:],
                                lhsT=w1eff[t][:, 128 * ht : 128 * (ht + 1)],
                                rhs=xslice(s, t),
                                start=(t == 0),
                                stop=(t == NQT - 1),
                            )
                    h = hp.tile(
                        [128, SB], bf16, tag=f"h{s}_{ht}", name=f"h{s}_{ht}"
                    )
                    if s == NSB - 1:
                        # last superblock: split the relu across two engines
                        # so the closing relu (which gates the last L2) is
                        # half as long
                        half = SB // 2
                        nc.scalar.activation(
                            h[:, :half],
                            ps1s[ht][:, :half],
                            AF.Relu,
                            bias=b1[:, ht : ht + 1],
                            scale=1.0,
                        )
                        nc.vector.tensor_scalar(
                            h[:, half:],
                            ps1s[ht][:, half:],
                            b1[:, ht : ht + 1],
                            0.0,
                            mybir.AluOpType.add,
                            mybir.AluOpType.max,
                        )
                    else:
                        nc.scalar.activation(
                            h[:],
                            ps1s[ht][:],
                            AF.Relu,
                            bias=b1[:, ht : ht + 1],
                            scale=1.0,
                        )
                    hs_all[(s, ht)] = h

            def l2_block(s):
                # bias-add split DVE/ACT; logits DMA out right after
                ps2 = ps2p.tile([NCLS, SB], f32, tag="ps2a", name=f"ps2_{s}")
                for ht in range(NHT):
                    nc.tensor.matmul(
                        ps2[:],
                        lhsT=w2[:, NCLS * ht : NCLS * (ht + 1)],
                        rhs=hs_all[(s, ht)][:],
                        start=(ht == 0),
                        stop=(ht == NHT - 1),
                    )
                half = SB // 2
                lo = SB * s
                nc.vector.tensor_scalar(
                    lg[:, lo : lo + half],
                    ps2[:, :half],
                    b2[:, 0:1],
                    None,
                    mybir.AluOpType.add,
                )
                nc.scalar.activation(
                    lg[:, lo + half : lo + SB],
                    ps2[:, half:],
                    AF.Identity,
                    bias=b2[:, 0:1],
                    scale=1.0,
                )
                eng = nc.sync if s % 2 == 0 else nc.scalar
                eng.dma_start(out=out_d[:, lo : lo + SB], in_=lg[:, lo : lo + SB])

            def l2_last():
                # s=3 in two half-N pieces so the closing chain
                # (relu -> 4 matmuls -> bias -> DMA) is as short as possible,
                # with the halves' bias+DMA on independent engine pairs.
                s = NSB - 1
                half = SB // 2
                lo = SB * s
                psa = ps2p.tile([NCLS, half], f32, tag="ps2a", name="ps2_3a")
                psb = ps2p.tile([NCLS, half], f32, tag="ps2b", name="ps2_3b")
                for ht in range(NHT):
                    nc.tensor.matmul(
                        psa[:],
                        lhsT=w2[:, NCLS * ht : NCLS * (ht + 1)],
                        rhs=hs_all[(s, ht)][:, :half],
                        start=(ht == 0),
                        stop=(ht == NHT - 1),
                    )
                nc.vector.tensor_scalar(
                    lg[:, lo : lo + half],
                    psa[:],
                    b2[:, 0:1],
                    None,
                    mybir.AluOpType.add,
                )
                nc.sync.dma_start(
                    out=out_d[:, lo : lo + half], in_=lg[:, lo : lo + half]
                )
                for ht in range(NHT):
                    nc.tensor.matmul(
                        psb[:],
                        lhsT=w2[:, NCLS * ht : NCLS * (ht + 1)],
                        rhs=hs_all[(s, ht)][:, half:],
                        start=(ht == 0),
                        stop=(ht == NHT - 1),
                    )
                nc.scalar.activation(
                    lg[:, lo + half : lo + SB],
                    psb[:],
                    AF.Identity,
                    bias=b2[:, 0:1],
                    scale=1.0,
                )
                nc.scalar.dma_start(
                    out=out_d[:, lo + half : lo + SB],
                    in_=lg[:, lo + half : lo + SB],
                )

            for t in range(NQT):
                fold_block(t)
            l1_block(0)
            l1_block(1)
            l2_block(0)
            l1_block(2)
            l2_block(1)
            l1_block(3)
            l2_block(2)
            l2_last()

    nc.compile()
    return nc


def _get_nc():
    if "nc" not in _CACHE:
        _CACHE["nc"] = _build()
    return _CACHE["nc"]


def kernel(x, conv_w, W1, b1, W2, b2):
    from concourse.bass_utils import run_bass_kernel_spmd

    nc = _get_nc()

    # C [784, 676]: y_flat = x @ C  (banded placement of conv_w values)
    C = np.zeros((Q, M26), dtype=np.float32)
    ii, jj = np.meshgrid(np.arange(OUT), np.arange(OUT), indexing="ij")
    m = (OUT * ii + jj).ravel()
    cw = np.asarray(conv_w, dtype=np.float32)
    for di in range(K):
        for dj in range(K):
            q = ((ii + di) * IMG + (jj + dj)).ravel()
            C[q, m] = cw[di, dj]
    CT = C.T  # [676, 784]
    # packed banded blocks, one [128, 112] block per pair, split by ring
    cm_pieces = []
    for lo, hi in _CM_SPLIT:
        piece = np.zeros((128, (hi - lo) * QT), dtype=np.float32)
        for k, p in enumerate(range(lo, hi)):
            t, mc = _PAIRS[p]
            rows = min(128, M26 - 128 * mc)
            piece[:rows, QT * k : QT * (k + 1)] = CT[
                128 * mc : 128 * mc + rows, QT * t : QT * (t + 1)
            ]
        cm_pieces.append(piece.astype(_BF16))

    # packed W1 m-chunks, split by ring
    w1f = np.asarray(W1, np.float32)
    w1_pieces = []
    for lo, hi in _W1_SPLIT:
        piece = np.zeros((128, (hi - lo) * HID), dtype=np.float32)
        for k, mc in enumerate(range(lo, hi)):
            rows = min(128, M26 - 128 * mc)
            piece[:rows, HID * k : HID * (k + 1)] = w1f[
                128 * mc : 128 * mc + rows, :
            ]
        w1_pieces.append(piece.astype(_BF16))

    b1l = np.ascontiguousarray(
        np.asarray(b1, np.float32).reshape(NHT, 128).T
    )  # [128, 4]
    w2l = np.ascontiguousarray(
        np.asarray(W2, np.float32)
        .reshape(NHT, 128, NCLS)
        .transpose(1, 0, 2)
        .reshape(128, NHT * NCLS)
    ).astype(_BF16)
    b2l = np.asarray(b2, np.float32).reshape(NCLS, 1)

    xf = np.asarray(x, np.float32)
    in_maps = []
    for c in range(NCORES):
        xt = np.ascontiguousarray(xf[c * BL : (c + 1) * BL].T).astype(_BF16)
        im = {"xt": xt, "b1l": b1l, "w2l": w2l, "b2l": b2l}
        for r in range(3):
            im[f"cm{r}"] = cm_pieces[r]
            im[f"w1{r}"] = w1_pieces[r]
        in_maps.append(im)

    kwargs = {}
    if TRACE:
        import profhook  # noqa: F401  (installs the NTFF hook shim)
        import tempfile

        kwargs = {"trace": True, "tmpdir": tempfile.mkdtemp(prefix="ntff_")}
    res = run_bass_kernel_spmd(nc, in_maps, core_ids=list(range(NCORES)), **kwargs)
    if TRACE:
        _CACHE["last_results"] = res

    out = np.concatenate(
        [np.ascontiguousarray(res.results[c]["out"].T) for c in range(NCORES)], axis=0
    ).astype(np.float32)
    return out
